# revision 24
# baseline (speedup 1.0000x reference)
"""Trainium2 Bass kernel for windowed per-channel sparse attention.

Problem shapes (hardcoded): B=4, REF=128, S=512, D=64, ET=128, H=4, ek=32, NH=128.

Algorithm (mathematically identical to the reference, never materializes the
[B,H,REF,S,D] softmax tensor):
  For each (b, r):
    scores[h,s] = q[h,r,:]·k[h,s,:]/sqrt(ek);  E[h,s] = exp(scores·scale - rowmax·scale)
    win[s,d]    = |tt[s]-qt[r]| <= stride[d]      (== reference's two-sided
                  comparison on the fp32 data; validated)
    X[s, (d|d)] = win ⊙ [mask*value | mask]       (one fused op per s-chunk,
                  3 chunks on DVE + 1 on GPSIMD)
    [numer|denom][d,h] = X.T @ E.T                (PE, fp32 PSUM accum)
    x[h,d] = numer/denom  (denom==0 -> mean_s value[s,d], matching softmax of an
             all-masked row degenerating to uniform 1/S)
  out[r,:] = x.flatten() @ Wo.T + bo

sigmoid computed as 1/(1+exp(-x)) so the ScalarE activation table never leaves
the exp set (a Sigmoid<->Exp switch costs 2x 1.28us table loads).

Sharding: 8 cores = (b, half-of-REF); core c handles b=c//2, r in [64*(c%2), +64).
"""

import sys
import numpy as np

sys.path.insert(0, "/opt/trn_rl_repo")

B, REF, S, D, ET, NH, H = 4, 128, 512, 64, 128, 128, 4
EK = ET // H          # 32
R = 64                # refs per core
NT = S // 128         # 4 s-chunks
SCALE = 1.0 / float(np.sqrt(np.float32(EK)))

_CACHE = {}


def _build():
    import concourse.bass as bass
    import concourse.tile as tile
    from concourse import bacc, mybir
    from concourse.masks import make_identity

    f32 = mybir.dt.float32
    AX = mybir.AxisListType
    OP = mybir.AluOpType
    AF = mybir.ActivationFunctionType
    PSUM = bass.MemorySpace.PSUM

    nc = bacc.Bacc("TRN2", target_bir_lowering=False, debug=False, num_devices=8)

    # wbig[128, 512]: [Wq | Wk | Wo]
    d_wbig = nc.dram_tensor("wbig", [128, 512], f32, kind="ExternalInput").ap()
    # wsmall[128, 199]: row 0 = [stride | stride | qt] (cols 0:192; stride is
    # host-computed sigmoid(si @ Wr.T + br) -- 8K MACs, negligible); query rows
    # 64:128 cols 0:128; tt chunks cols 192:196; bq 196; bk 197; bo 198
    d_wsmall = nc.dram_tensor("wsmall", [128, 199], f32, kind="ExternalInput").ap()
    d_key = nc.dram_tensor("key", [S, ET], f32, kind="ExternalInput").ap()
    # vm[128, t*128 + ([value chunk t | maskf32 chunk t])]
    d_vm = nc.dram_tensor("vm", [128, 512], f32, kind="ExternalInput").ap()
    d_out = nc.dram_tensor("out", [NH, R], f32, kind="ExternalOutput").ap()
    d_str = nc.dram_tensor("strideo", [1, D], f32, kind="ExternalOutput").ap()

    with tile.TileContext(nc) as tc:
        with (
            tc.tile_pool(name="persist", bufs=1) as pp,
            tc.tile_pool(name="xwork", bufs=36) as xp,
            tc.tile_pool(name="ework", bufs=2) as ewp,
            tc.tile_pool(name="ptrans", bufs=2, space=PSUM) as ptr,
            tc.tile_pool(name="pscore", bufs=2, space=PSUM) as psc,
            tc.tile_pool(name="pres", bufs=1, space=PSUM) as prs,
            tc.tile_pool(name="psmall", bufs=1, space=PSUM) as psm,
        ):
            # ---------- constants ----------
            ident = pp.tile([128, 128], f32, tag="ident")
            make_identity(nc, ident[:])
            ones1 = pp.tile([1, 128], f32, tag="ones1")
            nc.gpsimd.memset(ones1[:], 1.0)

            # ---------- input staging: 4 DMAs over both HWDGE queues ----------
            wsmall = pp.tile([128, 199], f32, tag="wsmall")
            nc.sync.dma_start(wsmall[:], d_wsmall)
            # MVcat4[s128, t*128 + (value[0:64] | maskf[64:128])]; value half is
            # overwritten in place by mask*value once vmean has been read out.
            MVcat4 = pp.tile([128, NT * 128], f32, tag="MVcat4")
            nc.scalar.dma_start(MVcat4[:], d_vm)
            key4 = pp.tile([128, S], f32, tag="key4")           # cols t*128+e
            nc.sync.dma_start(key4[:].rearrange("p (t e) -> p t e", t=NT),
                              d_key.rearrange("(t p) e -> p t e", p=128))
            wbig = pp.tile([128, 512], f32, tag="wbig")
            nc.scalar.dma_start(wbig[:], d_wbig)

            Wq_sb = wbig[:, 0:128]
            Wk_sb = wbig[:, 128:256]
            Wo_sb = wbig[:, 256:512]
            sc_row = wsmall[0:1, 0:128]
            stride_row = wsmall[0:1, 0:64]
            qt_row = wsmall[0:1, 128:192]
            query_sb = wsmall[64:128, 0:128]
            ttcols = wsmall[:, 192:192 + NT]
            bq_col = wsmall[:, 196:197]
            bk_col = wsmall[:, 197:198]
            bo_col = wsmall[:, 198:199]

            # ---------- transposes (PE + ScalarE evac) ----------
            def pe_transpose(dst_ap, src_ap, n):
                tp = ptr.tile([128, 128], f32, tag="tp")
                nc.tensor.transpose(tp[: src_ap.shape[1], : src_ap.shape[0]], src_ap, ident[:n, :n])
                nc.scalar.copy(dst_ap, tp[: src_ap.shape[1], : src_ap.shape[0]])

            # -- critical-path-first: SCb, cT, MVcat4 --
            sc_ps = ptr.tile([128, 128], f32, tag="tp")
            nc.tensor.matmul(sc_ps[:], ones1[:], sc_row, start=True, stop=True)
            SCb = pp.tile([128, 128], f32, tag="SCb")      # stridecat bcast down partitions
            nc.scalar.copy(SCb[:], sc_ps[:])

            qr_ps = ptr.tile([128, 128], f32, tag="tp")
            nc.tensor.matmul(qr_ps[:, :R], ones1[:], qt_row, start=True, stop=True)
            qt_rep = pp.tile([128, R], f32, tag="qt_rep")
            nc.scalar.copy(qt_rep[:], qr_ps[:, :R])

            # cT[s128, t*64 + r] = |tt[128t+s] - qt[r]|  (Abs is in every act table)
            cT = pp.tile([128, NT * R], f32, tag="cT")
            for t in range(NT):
                nc.scalar.activation(cT[:, t * R:(t + 1) * R], qt_rep[:], AF.Abs,
                                     bias=ttcols[:, t:t + 1], scale=-1.0)

            # vmean_col[d] = sum_s value[s, d] / S  (read raw value halves first)
            onesc = pp.tile([128, 1], f32, tag="onesc")
            nc.gpsimd.memset(onesc[:], 1.0)
            vs_ps = ptr.tile([128, 128], f32, tag="tp")
            for t in range(NT):
                nc.tensor.matmul(vs_ps[0:D, 0:1], MVcat4[:, t * 128:t * 128 + D], onesc[:],
                                 start=(t == 0), stop=(t == NT - 1))
            vmean_col = pp.tile([D, 1], f32, tag="vmean_col")
            nc.scalar.activation(vmean_col[:], vs_ps[0:D, 0:1], AF.Copy, scale=1.0 / S)

            # in-place: value half *= mask half  (WAR on the vmean matmuls is
            # tracked by Tile, so these wait for the reads above)
            for t in range(NT):
                nc.gpsimd.tensor_tensor(out=MVcat4[:, t * 128:t * 128 + D],
                                        in0=MVcat4[:, t * 128:t * 128 + D],
                                        in1=MVcat4[:, t * 128 + D:(t + 1) * 128], op=OP.mult)

            # ---------- projections / scores / E (feeds PE, trails DVE) ----------
            queryT = pp.tile([ET, R], f32, tag="queryT")
            tpq = ptr.tile([128, 128], f32, tag="tp")
            nc.tensor.transpose(tpq[:, :R], query_sb, ident[64:128, 64:128])
            nc.scalar.copy(queryT[:], tpq[:, :R])
            WqT = pp.tile([ET, ET], f32, tag="WqT")
            pe_transpose(WqT[:], Wq_sb, ET)
            WkT = pp.tile([ET, ET], f32, tag="WkT")
            pe_transpose(WkT[:], Wk_sb, ET)
            keyT = pp.tile([ET, S], f32, tag="keyT")
            for t in range(NT):
                pe_transpose(keyT[:, t * 128:(t + 1) * 128], key4[:, t * 128:(t + 1) * 128], 128)

            # qTh[ek, h*R + r], kTh[ek, h*S + s]
            qTh = pp.tile([EK, H * R], f32, tag="qTh")
            kTh = pp.tile([EK, H * S], f32, tag="kTh")
            for h in range(H):
                qh_ps = ptr.tile([128, 128], f32, tag="tp")
                nc.tensor.matmul(qh_ps[:EK, :R], WqT[:, EK * h:EK * (h + 1)], queryT[:],
                                 start=True, stop=True)
                nc.scalar.activation(qTh[:, h * R:(h + 1) * R], qh_ps[:EK, :R],
                                     AF.Identity, bias=bq_col[EK * h:EK * (h + 1), 0:1])
                kh_ps = psc.tile([128, S], f32, tag="pscore")
                nc.tensor.matmul(kh_ps[:EK, :], WkT[:, EK * h:EK * (h + 1)], keyT[:],
                                 start=True, stop=True)
                nc.scalar.activation(kTh[:, h * S:(h + 1) * S], kh_ps[:EK, :],
                                     AF.Identity, bias=bk_col[EK * h:EK * (h + 1), 0:1])

            # scores + E + ETt, emitted mid-pair-loop so the DVE reduce ops land
            # after ST psum is actually ready
            E = [None, None]
            ETt = pp.tile([128, NT * 256], f32, tag="ETt")

            def scores_section():
                    for j in range(2):
                        ST = psc.tile([128, S], f32, tag="pscore")
                        for hh in range(2):
                            h = 2 * j + hh
                            nc.tensor.matmul(ST[64 * hh:64 * (hh + 1), :],
                                             qTh[:, h * R:(h + 1) * R],
                                             kTh[:, h * S:(h + 1) * S], start=True, stop=True)
                        Mx = pp.tile([128, 1], f32, tag=f"Mx{j}")
                        nc.vector.tensor_reduce(out=Mx[:], in_=ST[:], axis=AX.X, op=OP.max)
                        Mn = pp.tile([128, 1], f32, tag=f"Mn{j}")
                        nc.vector.tensor_scalar_mul(Mn[:], Mx[:], -SCALE)
                        Ej = pp.tile([128, S], f32, tag=f"E{j}")
                        nc.scalar.activation(Ej[:], ST[:], AF.Exp, bias=Mn[:, 0:1], scale=SCALE)
                        E[j] = Ej

                    # ET[s128, 256t + 128j + (h%2)*64 + r]
                    for t in range(NT):
                        for j in range(2):
                            tp = ptr.tile([128, 128], f32, tag="tp")
                            nc.tensor.transpose(tp[:], E[j][:, t * 128:(t + 1) * 128], ident[:])
                            nc.scalar.copy(ETt[:, t * 256 + j * 128:t * 256 + (j + 1) * 128], tp[:])

            # ---------- pair loop (epilogue quarters interleaved) ----------
            # PR[q][(half,d), 4*rl + h]: rows 0:64 numer, 64:128 denom; 16 pairs/tile
            PR = [prs.tile([128, 64], f32, tag=f"PR{q}", name=f"PR{q}") for q in range(4)]
            xTT = [pp.tile([128, R], f32, tag=f"xTT{k}", name=f"xTT{k}") for k in range(2)]

            OPps = ptr.tile([128, 128], f32, tag="tp")
            out_sb = pp.tile([NH, R], f32, tag="out_sb")

            def outproj(j):
                # out.T[:, 32j:32j+32] = (x @ Wo.T + bo).T  (host un-transposes)
                sl = slice(32 * j, 32 * j + 32)
                nc.tensor.matmul(OPps[:, sl], WoT0[:], xTT[0][:, sl], start=True, stop=False)
                nc.tensor.matmul(OPps[:, sl], WoT1[:], xTT[1][:, sl], start=False, stop=True)
                nc.scalar.activation(out_sb[:, sl], OPps[:, sl], AF.Identity, bias=bo_col[:])
                nc.sync.dma_start(d_out[:, sl], out_sb[:, sl])

            # Mask-apply for one pair: TensorScalarPtr is DVE-only on HW, so
            # chunks 0,1 use the fused DVE op while chunks 2,3 split into a DVE
            # compare (2x-mode tensor_scalar) + a GPSIMD multiply.
            def build_X4(r):
                X4 = xp.tile([128, NT * 128], f32, tag="X4", name="X4")
                W2 = xp.tile([128, 128], f32, tag="W2", name="W2")
                for t in (0, 1):
                    nc.vector.scalar_tensor_tensor(
                        out=X4[:, t * 128:(t + 1) * 128], in0=SCb[:],
                        scalar=cT[:, t * R + r:t * R + r + 1],
                        in1=MVcat4[:, t * 128:(t + 1) * 128],
                        op0=OP.is_ge, op1=OP.mult)
                for t in (2, 3):
                    nc.vector.tensor_scalar(
                        out=W2[:, (t - 2) * 64:(t - 1) * 64], in0=SCb[:, 0:D],
                        scalar1=cT[:, t * R + r:t * R + r + 1], scalar2=None,
                        op0=OP.is_ge)
                # one merged GPSIMD multiply for both split chunks; the win mask
                # is read twice per chunk via a 0-stride AP (value|mask halves)
                w_rep = W2[:, 0:128].rearrange("p (c d) -> p c d", c=2)[:, :, None, :].broadcast_to((128, 2, 2, 64))
                nc.gpsimd.tensor_tensor(out=X4[:, 256:512].rearrange("p (c h d) -> p c h d", c=2, h=2),
                                        in0=w_rep,
                                        in1=MVcat4[:, 256:512].rearrange("p (c h d) -> p c h d", c=2, h=2),
                                        op=OP.mult)
                return X4

            # Emit the first PRELUDE pairs' mask ops before scores_section so the
            # scheduler places the DVE reduces after ~16 pairs of streaming.
            PRELUDE = 16
            X4pre = [build_X4(r) for r in range(PRELUDE)]
            scores_section()

            def epilogue(q):
                # x = numer/denom, with denom==0 -> vmean (uniform-softmax fallback)
                iz = ewp.tile([D, 64], f32, tag="iz", name="iz")
                nc.vector.tensor_scalar(out=iz[:], in0=PR[q][D:2 * D, :], scalar1=0.0,
                                        scalar2=None, op0=OP.is_equal)
                den2 = ewp.tile([D, 64], f32, tag="den2", name="den2")
                nc.vector.tensor_tensor(out=den2[:], in0=PR[q][D:2 * D, :], in1=iz[:], op=OP.add)
                rec = ewp.tile([D, 64], f32, tag="rec", name="rec")
                nc.vector.reciprocal(rec[:], den2[:])
                num2 = ewp.tile([D, 64], f32, tag="num2", name="num2")
                nc.vector.scalar_tensor_tensor(out=num2[:], in0=iz[:], scalar=vmean_col[:, 0:1],
                                               in1=PR[q][0:D, :], op0=OP.mult, op1=OP.add)
                if q < 3:
                    xt = ewp.tile([D, 64], f32, tag="xt", name="xt")
                    nc.vector.tensor_tensor(out=xt[:], in0=num2[:], in1=rec[:], op=OP.mult)
                    # xt[d, 4*rlq + h] -> xTT[h//2] rows (h%2)*D+d, cols 16q + rlq
                    xtr = xt[:].rearrange("d (r h) -> d h r", h=4)
                    for h in range(4):
                        nc.scalar.copy(xTT[h // 2][(h % 2) * D:(h % 2 + 1) * D, 16 * q:16 * (q + 1)],
                                       xtr[:, h, :])
                else:
                    # tail quarter: write xTT directly from DVE (it is idle here)
                    n2r = num2[:].rearrange("d (r h) -> d h r", h=4)
                    rcr = rec[:].rearrange("d (r h) -> d h r", h=4)
                    for h in range(4):
                        nc.vector.tensor_tensor(
                            out=xTT[h // 2][(h % 2) * D:(h % 2 + 1) * D, 16 * q:16 * (q + 1)],
                            in0=n2r[:, h, :], in1=rcr[:, h, :], op=OP.mult)

            for r in range(R):
                j, rl = r // 16, r % 16
                X4 = X4pre[r] if r < PRELUDE else build_X4(r)
                for t in range(NT):
                    rhs = ETt[:, t * 256:(t + 1) * 256].rearrange("p (h r) -> p h r", r=R)[:, :, r]
                    nc.tensor.matmul(PR[j][:, 4 * rl:4 * rl + 4],
                                     X4[:, t * 128:(t + 1) * 128], rhs,
                                     start=(t == 0), stop=(t == NT - 1))
                if r == 8:
                    WoT0 = pp.tile([128, NH], f32, tag="WoT0")
                    pe_transpose(WoT0[:], Wo_sb[:, 0:128], NH)
                    WoT1 = pp.tile([128, NH], f32, tag="WoT1")
                    pe_transpose(WoT1[:], Wo_sb[:, 128:256], NH)
                if r in (44, 52, 60):
                    epilogue((r - 44) // 8)
                if r == 56:
                    outproj(0)
            epilogue(3)
            outproj(1)

            nc.scalar.dma_start(d_str, stride_row)

    nc.compile()
    return nc


def _get_nc():
    if "nc" not in _CACHE:
        _CACHE["nc"] = _build()
    return _CACHE["nc"]


def make_in_maps(query, key_t, value, mask, qt, tt, stride_in, Wq, bq, Wk, bk, Wo, bo, Wr, br):
    def f(x, dtype=np.float32):
        return np.ascontiguousarray(np.asarray(x, dtype=dtype))
    in_maps = []
    for c in range(8):
        b, rh = c // 2, c % 2
        rs = slice(R * rh, R * rh + R)
        wbig = np.zeros((128, 512), np.float32)
        wbig[:, 0:128] = Wq
        wbig[:, 128:256] = Wk
        wbig[:, 256:512] = Wo
        si32 = np.asarray(stride_in[b], np.float32).reshape(64)
        pre = (si32 @ np.asarray(Wr, np.float32).T + np.asarray(br, np.float32)).astype(np.float32)
        stride_host = (np.float32(1.0) / (np.float32(1.0) + np.exp(-pre, dtype=np.float32))).astype(np.float32)
        wsmall = np.zeros((128, 199), np.float32)
        wsmall[0, 0:64] = stride_host
        wsmall[0, 64:128] = stride_host
        wsmall[0, 128:192] = qt[rs]
        wsmall[64:128, 0:128] = query[b, rs]
        wsmall[:, 192:196] = np.asarray(tt[b], np.float32).reshape(4, 128).T
        wsmall[:, 196] = bq
        wsmall[:, 197] = bk
        wsmall[:, 198] = bo
        vm = np.zeros((128, 512), np.float32)
        vf = np.asarray(value[b], np.float32).reshape(4, 128, 64)
        mf = np.asarray(mask[b] != 0, np.float32).reshape(4, 128, 64)
        for t in range(4):
            vm[:, t * 128:t * 128 + 64] = vf[t]
            vm[:, t * 128 + 64:(t + 1) * 128] = mf[t]
        in_maps.append({
            "wbig": f(wbig), "wsmall": f(wsmall), "vm": f(vm),
            "key": f(key_t[b]),
        })
    return in_maps


def kernel(query, key_t, value, mask, qt, tt, stride_in, Wq, bq, Wk, bk, Wo, bo, Wr, br):
    from concourse.bass_utils import run_bass_kernel_spmd

    nc = _get_nc()
    in_maps = make_in_maps(query, key_t, value, mask, qt, tt, stride_in,
                           Wq, bq, Wk, bk, Wo, bo, Wr, br)
    res = run_bass_kernel_spmd(nc, in_maps, core_ids=list(range(8)))
    out = np.zeros((B, REF, NH), np.float32)
    stride = np.zeros((B, 1, 1, D), np.float32)
    for c in range(8):
        b, rh = c // 2, c % 2
        out[b, R * rh:R * rh + R] = res.results[c]["out"].T
        if rh == 0:
            stride[b, 0, 0] = res.results[c]["strideo"][0]
    return out, stride
